# revision 20
# baseline (speedup 1.0000x reference)
"""Trainium2 Bass kernel for nn_Listener (LSTM listener + dense encoders). v2.1

Reference computation (per full batch B=512):
    emb = embed_table[message]                       # [B, T, 512]
    LSTM over T=128 steps, HIDDEN=1024:
        gated = [x_t, h] @ W_cell + b_cell           # [B, 4096] (i, g, f, o)
        f = sigmoid(f + 1); c = f*c + sigmoid(i)*tanh(g); h = sigmoid(o)*tanh(c)
    images_encoded = images @ W_img + b_img          # [B, 1024]
    hidden_encoded = h @ W_hid + b_hid               # [B, 1024]

Strategy (8 NeuronCores, data-parallel over batch, 64 rows/core):
  * Embedding + x-projection folded into a gathered table M2 (per token,
    per partition-half), injected into PSUM by full-partition identity
    matmuls that also seed the accumulation groups (start=True).
  * PSUM bank j holds gates [i | 2g | f+1 | o] (128 cols each) for unit
    slice U_j (256 units: lo half on partitions 0:64, hi on 64:128). Two
    sigmoid activations per bank compute every gate nonlinearity
    (tanh(x) = 2*sigmoid(2x)-1, the 2x folded into weights host-side).
  * h-pass: col-tiled matmul pairs (two 64-col groups concurrent),
    bank-pair-outer/chunk-inner order -> 216 ns/pair (streaming bound);
    banks 0,1 finish at half-stream so their epilogues hide under banks
    2,3's matmuls.
  * h -> hT with PE transposes interleaved into the stream; issue order
    keeps the DVE FIFO free of head-of-line blocking (hT casts issued
    before the late slices' epilogues).
  * Banks 2,3 double-buffered by step parity so next-step X seeds never
    wait; banks 0,1 single-buffered (their activations run mid-stream).
"""

import numpy as np

B, T = 512, 128
HIDDEN = 1024
VOCAB = 1024
EMB = 512
OUT = 1024
D_IMG = 2048
NCORES = 8
BS = B // NCORES  # 64 batch rows per core

_CACHE = {}


def _build_nc(n_steps: int):
    import concourse.bass as bass
    import concourse.mybir as mybir
    from concourse import bacc, tile

    f32 = mybir.dt.float32
    f32r = mybir.dt.float32r
    bf16 = mybir.dt.bfloat16
    i32 = mybir.dt.int32
    AF = mybir.ActivationFunctionType

    nc = bacc.Bacc("TRN2", target_bir_lowering=False, debug=False)

    m2p_d = nc.declare_dram_parameter("m2p", [2 * VOCAB, 2048], bf16, isOutput=False)
    w2_d = nc.declare_dram_parameter("w2", [HIDDEN, 4096], bf16, isOutput=False)
    msg2_d = nc.declare_dram_parameter("msg2", [128, T], i32, isOutput=False)
    identb_d = nc.declare_dram_parameter("identb", [128, 128], bf16, isOutput=False)
    imt_d = nc.declare_dram_parameter("imt", [128, D_IMG // 2], bf16, isOutput=False)
    wimg_d = nc.declare_dram_parameter("wimg", [D_IMG, OUT], bf16, isOutput=False)
    whid_d = nc.declare_dram_parameter("whid", [HIDDEN, OUT], bf16, isOutput=False)
    o2_d = nc.declare_dram_parameter("o2", [2, 128], f32r, isOutput=False)
    bimg2_d = nc.declare_dram_parameter("bimg2", [2, OUT // 2], f32r, isOutput=False)
    bhid2_d = nc.declare_dram_parameter("bhid2", [2, OUT // 2], f32r, isOutput=False)
    oimg_d = nc.declare_dram_parameter("oimg", [128, OUT // 2], f32, isOutput=True)
    ohid_d = nc.declare_dram_parameter("ohid", [128, OUT // 2], f32, isOutput=True)

    with tile.TileContext(nc) as tc:
        with (
            tc.tile_pool(name="wpool", bufs=1) as wpool,
            tc.tile_pool(name="const", bufs=1) as cpool,
            tc.tile_pool(name="xg", bufs=2) as xgpool,
            tc.tile_pool(name="state", bufs=2) as stpool,
            tc.tile_pool(name="act", bufs=2) as apool,
            tc.tile_pool(name="outs", bufs=1) as opool,
        ):
            # ---- constants / small inputs ----
            msg2 = cpool.tile([128, T], i32, tag="msg2")
            nc.sync.dma_start(msg2[:], msg2_d[:])
            identb = cpool.tile([128, 128], bf16, tag="identb")
            nc.sync.dma_start(identb[:], identb_d[:])
            o2 = cpool.tile([2, 128], f32r, tag="o2")
            nc.sync.dma_start(o2[:], o2_d[:])
            bimg2 = cpool.tile([2, OUT // 2], f32r, tag="bimg2")
            nc.sync.dma_start(bimg2[:], bimg2_d[:])
            bhid2 = cpool.tile([2, OUT // 2], f32r, tag="bhid2")
            nc.sync.dma_start(bhid2[:], bhid2_d[:])

            # ---- W2 resident in SBUF, split by bank pair and DMA'd in
            # need-order (bankpair01 cols first) to soften early-step stalls
            w2a_sb, w2b_sb = [], []
            for q in range(8):
                wt = wpool.tile([128, 2048], bf16, tag=f"w2a{q}")
                nc.sync.dma_start(wt[:], w2_d[128 * q : 128 * (q + 1), 0:2048])
                w2a_sb.append(wt)
            for q in range(8):
                wt = wpool.tile([128, 2048], bf16, tag=f"w2b{q}")
                # second hwdge queue so w2a/w2b descriptor issues run in
                # parallel; W2 transfers then hit the HBM-bandwidth floor
                nc.scalar.dma_start(wt[:], w2_d[128 * q : 128 * (q + 1), 2048:4096])
                w2b_sb.append(wt)

            # encoder inputs/weights: tiles allocated now, DMAs deferred to
            # mid-loop so their 6MB doesn't steal HBM bandwidth from W2
            # during the startup-critical steps
            imt = cpool.tile([128, D_IMG // 2], bf16, tag="imt")
            wimg_sb = []
            for ci in range(16):
                wt = wpool.tile([128, OUT], bf16, tag=f"wimg{ci}",
                                name=f"wimg{ci}")
                wimg_sb.append(wt)
            whid_sb = []
            for ci in range(8):
                wt = wpool.tile([128, OUT], bf16, tag=f"whid{ci}",
                                name=f"whid{ci}")
                whid_sb.append(wt)

            def issue_encoder_dmas():
                nc.sync.dma_start(imt[:], imt_d[:])
                for ci in range(16):
                    nc.sync.dma_start(
                        wimg_sb[ci][:], wimg_d[128 * ci : 128 * (ci + 1), :]
                    )
                for ci in range(8):
                    nc.sync.dma_start(
                        whid_sb[ci][:], whid_d[128 * ci : 128 * (ci + 1), :]
                    )

            hT = [None] * 4       # per-slice hT tiles (chunks 2j, 2j+1)
            h_cur = [None] * 4    # h tiles awaiting transpose
            c_prev = [None] * 4
            gpb_prev = [None] * 4

            with (
                tc.tile_pool(name="pgates", bufs=1, space="PSUM") as pgpool,
                tc.tile_pool(name="ptp", bufs=2, space="PSUM") as tppool,
            ):
                def gp_tag(t, j):
                    return f"gp{j}" if j < 2 else f"gp{j}{t % 2}"

                def epilogue(t, j, gpb_j):
                    """Gates psum bank j -> S -> c,h for unit slice U_j."""
                    S = apool.tile([128, 512], f32, tag=f"S{j}", name=f"S{j}_{t}")
                    nc.scalar.activation(S[:, 256:512], gpb_j[:, 256:512], AF.Sigmoid)
                    nc.scalar.activation(S[:, 0:256], gpb_j[:, 0:256], AF.Sigmoid)
                    m1 = apool.tile([128, 128], f32, tag=f"m1{j}", name=f"m1{j}_{t}")
                    acc1 = apool.tile([128, 1], f32, tag=f"ac1{j}", name=f"ac1{j}_{t}")
                    # m1 = tanh(g)*sig(i) = (2*sig2g - 1)*sigi
                    nc.vector.affine_mul_reduce(
                        out=m1[:], accum_out=acc1[:],
                        in0=S[:, 128:256], in1=S[:, 0:128], scale=2.0, bias=-1.0,
                    )
                    c_new = stpool.tile([128, 128], f32, tag=f"c{j}", name=f"c{j}_{t}")
                    if t == 0:
                        nc.vector.tensor_copy(c_new[:], m1[:])
                    else:
                        cm = apool.tile([128, 128], f32, tag=f"cm{j}", name=f"cm{j}_{t}")
                        nc.vector.tensor_mul(cm[:], S[:, 256:384], c_prev[j][:])
                        nc.vector.tensor_add(c_new[:], cm[:], m1[:])
                    t2 = apool.tile([128, 128], f32, tag=f"t2{j}", name=f"t2{j}_{t}")
                    nc.scalar.activation(t2[:], c_new[:], AF.Sigmoid, scale=2.0)
                    h_j = apool.tile([128, 128], bf16, tag=f"h{j}", name=f"h{j}_{t}")
                    acc2 = apool.tile([128, 1], f32, tag=f"ac2{j}", name=f"ac2{j}_{t}")
                    # h = tanh(c)*sig(o) = (2*sig2c - 1)*sigo
                    nc.vector.affine_mul_reduce(
                        out=h_j[:], accum_out=acc2[:],
                        in0=t2[:], in1=S[:, 384:512], scale=2.0, bias=-1.0,
                    )
                    c_prev[j] = c_new
                    h_cur[j] = h_j

                def tr_cast(t, j):
                    """PE-transpose h slice j into fresh hT tile (bf16)."""
                    tp = tppool.tile([128, 1024], bf16, tag="tp", name=f"tp{j}_{t}")
                    nc.tensor.transpose(
                        out=tp[:, 0:128], in_=h_cur[j][:], identity=identb[:]
                    )
                    hT_j = stpool.tile([128, 128], bf16, tag=f"hT{j}",
                                       name=f"hT{j}_{t}")
                    nc.vector.tensor_copy(hT_j[:], tp[:, 0:128])
                    hT[j] = hT_j

                def hpass_chunk(t, q, jpair, gpb_t):
                    lhs = hT[q // 2][:, 64 * (q % 2) : 64 * (q % 2) + 64]
                    wt = w2a_sb[q] if jpair[0] == 0 else w2b_sb[q]
                    for j in jpair:
                        jin = j % 2
                        last = q == 7
                        nc.tensor.matmul(
                            out=gpb_t[j][0:64, :], lhsT=lhs,
                            rhs=wt[:, 512 * jin : 512 * jin + 512],
                            start=False, stop=last, skip_group_check=True,
                        )
                        nc.tensor.matmul(
                            out=gpb_t[j][64:128, :], lhsT=lhs,
                            rhs=wt[:, 1024 + 512 * jin : 1024 + 512 * jin + 512],
                            start=False, stop=last, skip_group_check=True,
                        )

                # PE warm-up: junk matmuls during the startup DMA window keep
                # the HAM activity monitor busy so the first real steps run at
                # 2.4 GHz instead of the cold 1.2 GHz. The output bank is
                # cleared by step 0's start=True X seed, so values are moot.
                wu = pgpool.tile([128, 512], f32, tag="gp0", name="warmup")
                for i in range(6):
                    nc.tensor.matmul(
                        out=wu[:, 0:128], lhsT=identb[:], rhs=identb[:],
                        start=True, stop=True, skip_group_check=True,
                    )
                for i in range(12):
                    nc.tensor.matmul(
                        out=wu[:], lhsT=identb[:], rhs=w2a_sb[0][:, 0:512],
                        start=True, stop=True, skip_group_check=True,
                    )

                for t in range(n_steps):
                    if t == 8:
                        issue_encoder_dmas()
                    # 1. gather x-contribution (double-buffered)
                    xg = xgpool.tile([128, 2048], bf16, tag="xg", name=f"xg_{t}")
                    nc.gpsimd.indirect_dma_start(
                        out=xg[:],
                        out_offset=None,
                        in_=m2p_d[:],
                        in_offset=bass.IndirectOffsetOnAxis(
                            ap=msg2[:, t : t + 1], axis=0
                        ),
                    )
                    # 2. X injection seeds banks 0,1 (full-partition identity
                    # mm); banks 2,3 seeded later, right before their h-pass
                    gpb = [
                        pgpool.tile([128, 512], f32, tag=gp_tag(t, j),
                                    name=f"gp{j}_{t}")
                        for j in range(4)
                    ]

                    def xseed(j):
                        nc.tensor.matmul(
                            out=gpb[j][:],
                            lhsT=identb[:],
                            rhs=xg[:, 512 * j : 512 * (j + 1)],
                            start=True,
                            stop=(t == 0),
                            skip_group_check=True,
                        )

                    xseed(0)
                    xseed(1)
                    if t == 0:
                        xseed(2)
                        xseed(3)
                    if t > 0:
                        # 3./4. transpose h slices 0,1 of step t-1 (DVE casts
                        # go FIRST in this region's DVE queue)
                        tr_cast(t - 1, 0)
                        tr_cast(t - 1, 1)
                        # 5. late epilogues for slices 2,3 of step t-1
                        epilogue(t - 1, 2, gpb_prev[2])
                        epilogue(t - 1, 3, gpb_prev[3])
                        # 6. h-pass bank pair (0,1), chunks 0..3
                        for q in range(4):
                            hpass_chunk(t, q, (0, 1), gpb)
                        # 7. transpose h slices 2,3 of step t-1
                        tr_cast(t - 1, 2)
                        tr_cast(t - 1, 3)
                        # 8. h-pass bank pair (0,1), chunks 4..7
                        for q in range(4, 8):
                            hpass_chunk(t, q, (0, 1), gpb)
                        # 8b. X seeds for banks 2,3 (their last-step activation
                        # readers are long done by now -> no PE stall)
                        xseed(2)
                        xseed(3)
                        # 9. h-pass bank pair (2,3), all chunks
                        for q in range(8):
                            hpass_chunk(t, q, (2, 3), gpb)
                    # 10. epilogues for slices 0,1 of step t
                    epilogue(t, 0, gpb[0])
                    epilogue(t, 1, gpb[1])
                    gpb_prev = gpb

                # ---- final step leftovers + encoders ----
                # The images encoder is independent of the LSTM state, so it
                # runs FIRST in the PE stream, covering the last step's
                # epilogue chain; its PSUM accumulator reuses the step-126
                # parity bank (free since its activations ran during step
                # 127's stream) -- no extra PSUM pool needed.
                tl = n_steps - 1
                epilogue(tl, 2, gpb_prev[2])
                epilogue(tl, 3, gpb_prev[3])

                # images encoder: out = images @ W_img + b_img
                oip = pgpool.tile([128, OUT // 2], f32, tag=gp_tag(tl - 1, 2),
                                  name="oip")
                nc.tensor.matmul(
                    out=oip[:], lhsT=o2[:], rhs=bimg2[:],
                    start=True, stop=False, skip_group_check=True,
                )
                for ci in range(16):
                    lhs = imt[:, 64 * ci : 64 * ci + 64]
                    last = ci == 15
                    nc.tensor.matmul(
                        out=oip[0:64, :], lhsT=lhs, rhs=wimg_sb[ci][:, 0:512],
                        start=False, stop=last, skip_group_check=True,
                    )
                    nc.tensor.matmul(
                        out=oip[64:128, :], lhsT=lhs, rhs=wimg_sb[ci][:, 512:1024],
                        start=False, stop=last, skip_group_check=True,
                    )

                # transposes of the last h (chain now hidden under the
                # images encoder matmuls); their DVE casts are issued before
                # the oimg copy so they can't be head-of-line blocked
                for j in range(4):
                    tr_cast(tl, j)
                oimg_sb = opool.tile([128, OUT // 2], f32, tag="oimg")
                nc.vector.tensor_copy(oimg_sb[:], oip[:])
                nc.sync.dma_start(oimg_d[:], oimg_sb[:])

                # hidden encoder: out = h @ W_hid + b_hid
                ohp = pgpool.tile([128, OUT // 2], f32, tag=gp_tag(tl - 1, 3),
                                  name="ohp")
                nc.tensor.matmul(
                    out=ohp[:], lhsT=o2[:], rhs=bhid2[:],
                    start=True, stop=False, skip_group_check=True,
                )
                for q in range(8):
                    lhs = hT[q // 2][:, 64 * (q % 2) : 64 * (q % 2) + 64]
                    last = q == 7
                    nc.tensor.matmul(
                        out=ohp[0:64, :], lhsT=lhs, rhs=whid_sb[q][:, 0:512],
                        start=False, stop=last, skip_group_check=True,
                    )
                    nc.tensor.matmul(
                        out=ohp[64:128, :], lhsT=lhs, rhs=whid_sb[q][:, 512:1024],
                        start=False, stop=last, skip_group_check=True,
                    )
                ohid_sb = opool.tile([128, OUT // 2], f32, tag="ohid")
                nc.vector.tensor_copy(ohid_sb[:], ohp[:])
                nc.sync.dma_start(ohid_d[:], ohid_sb[:])

    nc.compile()
    return nc


def _host_prep(images, embed_table, W_cell, b_cell, W_img, b_img, W_hid, b_hid,
               message):
    """Builds the per-core input maps (all host-side preprocessing)."""
    from ml_dtypes import bfloat16

    W_x = W_cell[:EMB]                          # [512, 4096]
    W_h = np.ascontiguousarray(W_cell[EMB:])    # [1024, 4096] f32

    # Fold: f-gate +1 bias, g-gate x2 (tanh(g) = 2*sigmoid(2g) - 1)
    M2 = embed_table.astype(np.float32) @ W_x + b_cell  # [1024, 4096]
    M2[:, 2 * HIDDEN : 3 * HIDDEN] += 1.0
    M2[:, HIDDEN : 2 * HIDDEN] *= 2.0
    W_h2 = W_h.copy()
    W_h2[:, HIDDEN : 2 * HIDDEN] *= 2.0

    # column reorder [gate, chunk, 128] -> rows: [v, half] cols: [j, gate, 128]
    M2r = M2.reshape(VOCAB, 4, 4, 2, 128)       # [v, gate, j, half, 128]
    M2p = np.ascontiguousarray(
        M2r.transpose(0, 3, 2, 1, 4).reshape(2 * VOCAB, 2048)
    ).astype(bfloat16)
    # W2 cols: [jpair, half, j-in-pair, gate, 128] so each bank pair's
    # columns are one contiguous 2048-block (DMA'd in need-order)
    W2r = W_h2.reshape(HIDDEN, 4, 2, 2, 2, 128)  # [u, gate, jp, jin, half, o]
    W2 = np.ascontiguousarray(
        W2r.transpose(0, 2, 4, 3, 1, 5).reshape(HIDDEN, 4096)
    ).astype(bfloat16)

    identb = np.eye(128, dtype=bfloat16)

    o2 = np.zeros((2, 128), np.float32)
    o2[0, 0:64] = 1.0
    o2[1, 64:128] = 1.0

    W_img_b = W_img.astype(bfloat16)
    W_hid_b = W_hid.astype(bfloat16)
    bimg2 = np.stack([b_img[: OUT // 2], b_img[OUT // 2 :]]).astype(np.float32)
    bhid2 = np.stack([b_hid[: OUT // 2], b_hid[OUT // 2 :]]).astype(np.float32)

    in_maps = []
    for core in range(NCORES):
        sl = slice(core * BS, (core + 1) * BS)
        msg = message[sl]  # [64, T] int32
        msg2 = np.concatenate([2 * msg, 2 * msg + 1], axis=0).astype(np.int32)
        # imt: [128 parts, 16*64]: chunk c at cols 64c, imt[p, 64c+m] =
        # images[sl][m, 128c+p]
        imc = images[sl].astype(np.float32)              # [64, 2048]
        imt = np.ascontiguousarray(
            imc.reshape(BS, 16, 128).transpose(2, 1, 0).reshape(128, 1024)
        ).astype(bfloat16)
        in_maps.append(
            {
                "m2p": M2p,
                "w2": W2,
                "msg2": msg2,
                "identb": identb,
                "imt": imt,
                "wimg": W_img_b,
                "whid": W_hid_b,
                "o2": o2,
                "bimg2": bimg2,
                "bhid2": bhid2,
            }
        )
    return in_maps


def kernel(images, embed_table, W_cell, b_cell, W_img, b_img, W_hid, b_hid,
           message):
    import sys
    if "/opt/trn_rl_repo" not in sys.path:
        sys.path.insert(0, "/opt/trn_rl_repo")
    from concourse.bass_utils import run_bass_kernel_spmd

    images = np.asarray(images, np.float32)
    embed_table = np.asarray(embed_table, np.float32)
    W_cell = np.asarray(W_cell, np.float32)
    b_cell = np.asarray(b_cell, np.float32)
    W_img = np.asarray(W_img, np.float32)
    b_img = np.asarray(b_img, np.float32)
    W_hid = np.asarray(W_hid, np.float32)
    b_hid = np.asarray(b_hid, np.float32)
    message = np.asarray(message, np.int32)

    n_steps = T
    if "nc" not in _CACHE or _CACHE.get("n_steps") != n_steps:
        _CACHE["nc"] = _build_nc(n_steps)
        _CACHE["n_steps"] = n_steps
    nc = _CACHE["nc"]

    in_maps = _host_prep(
        images, embed_table, W_cell, b_cell, W_img, b_img, W_hid, b_hid, message
    )
    res = run_bass_kernel_spmd(nc, in_maps, core_ids=list(range(NCORES)))
    results = res.results

    images_encoded = np.empty((B, OUT), np.float32)
    hidden_encoded = np.empty((B, OUT), np.float32)
    for core in range(NCORES):
        sl = slice(core * BS, (core + 1) * BS)
        oi = results[core]["oimg"]
        oh = results[core]["ohid"]
        images_encoded[sl, : OUT // 2] = oi[0:64]
        images_encoded[sl, OUT // 2 :] = oi[64:128]
        hidden_encoded[sl, : OUT // 2] = oh[0:64]
        hidden_encoded[sl, OUT // 2 :] = oh[64:128]
    return images_encoded, hidden_encoded


# revision 22
# speedup vs baseline: 1.0013x; 1.0013x over previous
"""Trainium2 Bass kernel for nn_Listener (LSTM listener + dense encoders). v2.1

Reference computation (per full batch B=512):
    emb = embed_table[message]                       # [B, T, 512]
    LSTM over T=128 steps, HIDDEN=1024:
        gated = [x_t, h] @ W_cell + b_cell           # [B, 4096] (i, g, f, o)
        f = sigmoid(f + 1); c = f*c + sigmoid(i)*tanh(g); h = sigmoid(o)*tanh(c)
    images_encoded = images @ W_img + b_img          # [B, 1024]
    hidden_encoded = h @ W_hid + b_hid               # [B, 1024]

Strategy (8 NeuronCores, data-parallel over batch, 64 rows/core):
  * Embedding + x-projection folded into a gathered table M2 (per token,
    per partition-half), injected into PSUM by full-partition identity
    matmuls that also seed the accumulation groups (start=True).
  * PSUM bank j holds gates [i | 2g | f+1 | o] (128 cols each) for unit
    slice U_j (256 units: lo half on partitions 0:64, hi on 64:128). Two
    sigmoid activations per bank compute every gate nonlinearity
    (tanh(x) = 2*sigmoid(2x)-1, the 2x folded into weights host-side).
  * h-pass: col-tiled matmul pairs (two 64-col groups concurrent),
    bank-pair-outer/chunk-inner order -> 216 ns/pair (streaming bound);
    banks 0,1 finish at half-stream so their epilogues hide under banks
    2,3's matmuls.
  * h -> hT with PE transposes interleaved into the stream; issue order
    keeps the DVE FIFO free of head-of-line blocking (hT casts issued
    before the late slices' epilogues).
  * Banks 2,3 double-buffered by step parity so next-step X seeds never
    wait; banks 0,1 single-buffered (their activations run mid-stream).
"""

import numpy as np

B, T = 512, 128
HIDDEN = 1024
VOCAB = 1024
EMB = 512
OUT = 1024
D_IMG = 2048
NCORES = 8
BS = B // NCORES  # 64 batch rows per core

_CACHE = {}


def _build_nc(n_steps: int):
    import concourse.bass as bass
    import concourse.mybir as mybir
    from concourse import bacc, tile

    f32 = mybir.dt.float32
    f32r = mybir.dt.float32r
    bf16 = mybir.dt.bfloat16
    i32 = mybir.dt.int32
    AF = mybir.ActivationFunctionType

    nc = bacc.Bacc("TRN2", target_bir_lowering=False, debug=False)

    m2p_d = nc.declare_dram_parameter("m2p", [2 * VOCAB, 2048], bf16, isOutput=False)
    w2_d = nc.declare_dram_parameter("w2", [HIDDEN, 4096], bf16, isOutput=False)
    msg2_d = nc.declare_dram_parameter("msg2", [128, T], i32, isOutput=False)
    identb_d = nc.declare_dram_parameter("identb", [128, 128], bf16, isOutput=False)
    imt_d = nc.declare_dram_parameter("imt", [128, D_IMG // 2], bf16, isOutput=False)
    wimg_d = nc.declare_dram_parameter("wimg", [D_IMG, OUT], bf16, isOutput=False)
    whid_d = nc.declare_dram_parameter("whid", [HIDDEN, OUT], bf16, isOutput=False)
    o2_d = nc.declare_dram_parameter("o2", [2, 128], f32r, isOutput=False)
    bimg2_d = nc.declare_dram_parameter("bimg2", [2, OUT // 2], f32r, isOutput=False)
    bhid2_d = nc.declare_dram_parameter("bhid2", [2, OUT // 2], f32r, isOutput=False)
    oimg_d = nc.declare_dram_parameter("oimg", [128, OUT // 2], f32, isOutput=True)
    ohid_d = nc.declare_dram_parameter("ohid", [128, OUT // 2], f32, isOutput=True)

    with tile.TileContext(nc) as tc:
        with (
            tc.tile_pool(name="wpool", bufs=1) as wpool,
            tc.tile_pool(name="const", bufs=1) as cpool,
            tc.tile_pool(name="xg", bufs=2) as xgpool,
            tc.tile_pool(name="state", bufs=2) as stpool,
            tc.tile_pool(name="act", bufs=2) as apool,
            tc.tile_pool(name="outs", bufs=1) as opool,
        ):
            # ---- constants / small inputs ----
            msg2 = cpool.tile([128, T], i32, tag="msg2")
            nc.sync.dma_start(msg2[:], msg2_d[:])
            identb = cpool.tile([128, 128], bf16, tag="identb")
            nc.sync.dma_start(identb[:], identb_d[:])
            o2 = cpool.tile([2, 128], f32r, tag="o2")
            nc.sync.dma_start(o2[:], o2_d[:])
            bimg2 = cpool.tile([2, OUT // 2], f32r, tag="bimg2")
            nc.sync.dma_start(bimg2[:], bimg2_d[:])
            bhid2 = cpool.tile([2, OUT // 2], f32r, tag="bhid2")
            nc.sync.dma_start(bhid2[:], bhid2_d[:])

            # ---- W2 resident in SBUF, split by bank pair and DMA'd in
            # need-order (bankpair01 cols first) to soften early-step stalls
            w2a_sb, w2b_sb = [], []
            for q in range(8):
                wt = wpool.tile([128, 2048], bf16, tag=f"w2a{q}")
                nc.sync.dma_start(wt[:], w2_d[128 * q : 128 * (q + 1), 0:2048])
                w2a_sb.append(wt)
            for q in range(8):
                wt = wpool.tile([128, 2048], bf16, tag=f"w2b{q}")
                nc.sync.dma_start(wt[:], w2_d[128 * q : 128 * (q + 1), 2048:4096])
                w2b_sb.append(wt)

            # ---- images (host-pretransposed) + encoder weights prefetch ----
            imt = cpool.tile([128, D_IMG // 2], bf16, tag="imt")
            nc.sync.dma_start(imt[:], imt_d[:])
            wimg_sb = []
            for ci in range(16):
                wt = wpool.tile([128, OUT], bf16, tag=f"wimg{ci}")
                nc.sync.dma_start(wt[:], wimg_d[128 * ci : 128 * (ci + 1), :])
                wimg_sb.append(wt)
            whid_sb = []
            for ci in range(8):
                wt = wpool.tile([128, OUT], bf16, tag=f"whid{ci}")
                nc.sync.dma_start(wt[:], whid_d[128 * ci : 128 * (ci + 1), :])
                whid_sb.append(wt)

            hT = [None] * 4       # per-slice hT tiles (chunks 2j, 2j+1)
            h_cur = [None] * 4    # h tiles awaiting transpose
            c_prev = [None] * 4
            gpb_prev = [None] * 4

            with (
                tc.tile_pool(name="pgates", bufs=1, space="PSUM") as pgpool,
                tc.tile_pool(name="ptp", bufs=2, space="PSUM") as tppool,
            ):
                def gp_tag(t, j):
                    return f"gp{j}" if j < 2 else f"gp{j}{t % 2}"

                def epilogue(t, j, gpb_j):
                    """Gates psum bank j -> S -> c,h for unit slice U_j."""
                    S = apool.tile([128, 512], f32, tag=f"S{j}", name=f"S{j}_{t}")
                    nc.scalar.activation(S[:, 256:512], gpb_j[:, 256:512], AF.Sigmoid)
                    nc.scalar.activation(S[:, 0:256], gpb_j[:, 0:256], AF.Sigmoid)
                    m1 = apool.tile([128, 128], f32, tag=f"m1{j}", name=f"m1{j}_{t}")
                    acc1 = apool.tile([128, 1], f32, tag=f"ac1{j}", name=f"ac1{j}_{t}")
                    # m1 = tanh(g)*sig(i) = (2*sig2g - 1)*sigi
                    nc.vector.affine_mul_reduce(
                        out=m1[:], accum_out=acc1[:],
                        in0=S[:, 128:256], in1=S[:, 0:128], scale=2.0, bias=-1.0,
                    )
                    c_new = stpool.tile([128, 128], f32, tag=f"c{j}", name=f"c{j}_{t}")
                    if t == 0:
                        nc.vector.tensor_copy(c_new[:], m1[:])
                    else:
                        cm = apool.tile([128, 128], f32, tag=f"cm{j}", name=f"cm{j}_{t}")
                        nc.vector.tensor_mul(cm[:], S[:, 256:384], c_prev[j][:])
                        nc.vector.tensor_add(c_new[:], cm[:], m1[:])
                    t2 = apool.tile([128, 128], f32, tag=f"t2{j}", name=f"t2{j}_{t}")
                    nc.scalar.activation(t2[:], c_new[:], AF.Sigmoid, scale=2.0)
                    h_j = apool.tile([128, 128], bf16, tag=f"h{j}", name=f"h{j}_{t}")
                    acc2 = apool.tile([128, 1], f32, tag=f"ac2{j}", name=f"ac2{j}_{t}")
                    # h = tanh(c)*sig(o) = (2*sig2c - 1)*sigo
                    nc.vector.affine_mul_reduce(
                        out=h_j[:], accum_out=acc2[:],
                        in0=t2[:], in1=S[:, 384:512], scale=2.0, bias=-1.0,
                    )
                    c_prev[j] = c_new
                    h_cur[j] = h_j

                def tr_cast(t, j):
                    """PE-transpose h slice j into fresh hT tile (bf16)."""
                    tp = tppool.tile([128, 1024], bf16, tag="tp", name=f"tp{j}_{t}")
                    nc.tensor.transpose(
                        out=tp[:, 0:128], in_=h_cur[j][:], identity=identb[:]
                    )
                    hT_j = stpool.tile([128, 128], bf16, tag=f"hT{j}",
                                       name=f"hT{j}_{t}")
                    nc.vector.tensor_copy(hT_j[:], tp[:, 0:128])
                    hT[j] = hT_j

                def hpass_chunk(t, q, jpair, gpb_t):
                    lhs = hT[q // 2][:, 64 * (q % 2) : 64 * (q % 2) + 64]
                    wt = w2a_sb[q] if jpair[0] == 0 else w2b_sb[q]
                    for j in jpair:
                        jin = j % 2
                        last = q == 7
                        nc.tensor.matmul(
                            out=gpb_t[j][0:64, :], lhsT=lhs,
                            rhs=wt[:, 512 * jin : 512 * jin + 512],
                            start=False, stop=last, skip_group_check=True,
                        )
                        nc.tensor.matmul(
                            out=gpb_t[j][64:128, :], lhsT=lhs,
                            rhs=wt[:, 1024 + 512 * jin : 1024 + 512 * jin + 512],
                            start=False, stop=last, skip_group_check=True,
                        )

                # PE warm-up: junk matmuls during the startup DMA window keep
                # the HAM activity monitor busy so the first real steps run at
                # 2.4 GHz instead of the cold 1.2 GHz. The output bank is
                # cleared by step 0's start=True X seed, so values are moot.
                # (gated on the first W2 chunk: earlier identb-gated warmups
                # re-throttle in the gap before W2 lands; 8 cold N=512 mms
                # ~= the 3.4us HAM window, ending right as the first gather
                # data becomes ready)
                wu = pgpool.tile([128, 512], f32, tag="gp0", name="warmup")
                for i in range(8):
                    nc.tensor.matmul(
                        out=wu[:], lhsT=identb[:], rhs=w2a_sb[0][:, 0:512],
                        start=True, stop=True, skip_group_check=True,
                    )

                for t in range(n_steps):
                    # 1. gather x-contribution (double-buffered)
                    xg = xgpool.tile([128, 2048], bf16, tag="xg", name=f"xg_{t}")
                    nc.gpsimd.indirect_dma_start(
                        out=xg[:],
                        out_offset=None,
                        in_=m2p_d[:],
                        in_offset=bass.IndirectOffsetOnAxis(
                            ap=msg2[:, t : t + 1], axis=0
                        ),
                    )
                    # 2. X injection seeds banks 0,1 (full-partition identity
                    # mm); banks 2,3 seeded later, right before their h-pass
                    gpb = [
                        pgpool.tile([128, 512], f32, tag=gp_tag(t, j),
                                    name=f"gp{j}_{t}")
                        for j in range(4)
                    ]

                    def xseed(j):
                        nc.tensor.matmul(
                            out=gpb[j][:],
                            lhsT=identb[:],
                            rhs=xg[:, 512 * j : 512 * (j + 1)],
                            start=True,
                            stop=(t == 0),
                            skip_group_check=True,
                        )

                    xseed(0)
                    xseed(1)
                    if t == 0:
                        xseed(2)
                        xseed(3)
                    if t > 0:
                        # 3./4. transpose h slices 0,1 of step t-1 (DVE casts
                        # go FIRST in this region's DVE queue)
                        tr_cast(t - 1, 0)
                        tr_cast(t - 1, 1)
                        # 5. late epilogues for slices 2,3 of step t-1
                        epilogue(t - 1, 2, gpb_prev[2])
                        epilogue(t - 1, 3, gpb_prev[3])
                        # 6. h-pass bank pair (0,1), chunks 0..3
                        for q in range(4):
                            hpass_chunk(t, q, (0, 1), gpb)
                        # 7. transpose h slices 2,3 of step t-1
                        tr_cast(t - 1, 2)
                        tr_cast(t - 1, 3)
                        # 8. h-pass bank pair (0,1), chunks 4..7
                        for q in range(4, 8):
                            hpass_chunk(t, q, (0, 1), gpb)
                        # 8b. X seeds for banks 2,3 (their last-step activation
                        # readers are long done by now -> no PE stall)
                        xseed(2)
                        xseed(3)
                        # 9. h-pass bank pair (2,3), all chunks
                        for q in range(8):
                            hpass_chunk(t, q, (2, 3), gpb)
                    # 10. epilogues for slices 0,1 of step t
                    epilogue(t, 0, gpb[0])
                    epilogue(t, 1, gpb[1])
                    gpb_prev = gpb

                # ---- final step leftovers + encoders ----
                # The images encoder is independent of the LSTM state, so it
                # runs FIRST in the PE stream, covering the last step's
                # epilogue chain; its PSUM accumulator reuses the step-126
                # parity bank (free since its activations ran during step
                # 127's stream) -- no extra PSUM pool needed.
                tl = n_steps - 1
                epilogue(tl, 2, gpb_prev[2])
                epilogue(tl, 3, gpb_prev[3])

                # images encoder: out = images @ W_img + b_img
                oip = pgpool.tile([128, OUT // 2], f32, tag=gp_tag(tl - 1, 2),
                                  name="oip")
                nc.tensor.matmul(
                    out=oip[:], lhsT=o2[:], rhs=bimg2[:],
                    start=True, stop=False, skip_group_check=True,
                )
                for ci in range(16):
                    lhs = imt[:, 64 * ci : 64 * ci + 64]
                    last = ci == 15
                    nc.tensor.matmul(
                        out=oip[0:64, :], lhsT=lhs, rhs=wimg_sb[ci][:, 0:512],
                        start=False, stop=last, skip_group_check=True,
                    )
                    nc.tensor.matmul(
                        out=oip[64:128, :], lhsT=lhs, rhs=wimg_sb[ci][:, 512:1024],
                        start=False, stop=last, skip_group_check=True,
                    )

                # transposes of the last h (chain now hidden under the
                # images encoder matmuls); their DVE casts are issued before
                # the oimg copy so they can't be head-of-line blocked
                for j in range(4):
                    tr_cast(tl, j)
                oimg_sb = opool.tile([128, OUT // 2], f32, tag="oimg")
                nc.vector.tensor_copy(oimg_sb[:], oip[:])
                nc.sync.dma_start(oimg_d[:], oimg_sb[:])

                # hidden encoder: out = h @ W_hid + b_hid
                ohp = pgpool.tile([128, OUT // 2], f32, tag=gp_tag(tl - 1, 3),
                                  name="ohp")
                nc.tensor.matmul(
                    out=ohp[:], lhsT=o2[:], rhs=bhid2[:],
                    start=True, stop=False, skip_group_check=True,
                )
                for q in range(8):
                    lhs = hT[q // 2][:, 64 * (q % 2) : 64 * (q % 2) + 64]
                    last = q == 7
                    nc.tensor.matmul(
                        out=ohp[0:64, :], lhsT=lhs, rhs=whid_sb[q][:, 0:512],
                        start=False, stop=last, skip_group_check=True,
                    )
                    nc.tensor.matmul(
                        out=ohp[64:128, :], lhsT=lhs, rhs=whid_sb[q][:, 512:1024],
                        start=False, stop=last, skip_group_check=True,
                    )
                ohid_sb = opool.tile([128, OUT // 2], f32, tag="ohid")
                nc.vector.tensor_copy(ohid_sb[:], ohp[:])
                nc.sync.dma_start(ohid_d[:], ohid_sb[:])

    nc.compile()
    return nc


def _host_prep(images, embed_table, W_cell, b_cell, W_img, b_img, W_hid, b_hid,
               message):
    """Builds the per-core input maps (all host-side preprocessing)."""
    from ml_dtypes import bfloat16

    W_x = W_cell[:EMB]                          # [512, 4096]
    W_h = np.ascontiguousarray(W_cell[EMB:])    # [1024, 4096] f32

    # Fold: f-gate +1 bias, g-gate x2 (tanh(g) = 2*sigmoid(2g) - 1)
    M2 = embed_table.astype(np.float32) @ W_x + b_cell  # [1024, 4096]
    M2[:, 2 * HIDDEN : 3 * HIDDEN] += 1.0
    M2[:, HIDDEN : 2 * HIDDEN] *= 2.0
    W_h2 = W_h.copy()
    W_h2[:, HIDDEN : 2 * HIDDEN] *= 2.0

    # column reorder [gate, chunk, 128] -> rows: [v, half] cols: [j, gate, 128]
    M2r = M2.reshape(VOCAB, 4, 4, 2, 128)       # [v, gate, j, half, 128]
    M2p = np.ascontiguousarray(
        M2r.transpose(0, 3, 2, 1, 4).reshape(2 * VOCAB, 2048)
    ).astype(bfloat16)
    # W2 cols: [jpair, half, j-in-pair, gate, 128] so each bank pair's
    # columns are one contiguous 2048-block (DMA'd in need-order)
    W2r = W_h2.reshape(HIDDEN, 4, 2, 2, 2, 128)  # [u, gate, jp, jin, half, o]
    W2 = np.ascontiguousarray(
        W2r.transpose(0, 2, 4, 3, 1, 5).reshape(HIDDEN, 4096)
    ).astype(bfloat16)

    identb = np.eye(128, dtype=bfloat16)

    o2 = np.zeros((2, 128), np.float32)
    o2[0, 0:64] = 1.0
    o2[1, 64:128] = 1.0

    W_img_b = W_img.astype(bfloat16)
    W_hid_b = W_hid.astype(bfloat16)
    bimg2 = np.stack([b_img[: OUT // 2], b_img[OUT // 2 :]]).astype(np.float32)
    bhid2 = np.stack([b_hid[: OUT // 2], b_hid[OUT // 2 :]]).astype(np.float32)

    in_maps = []
    for core in range(NCORES):
        sl = slice(core * BS, (core + 1) * BS)
        msg = message[sl]  # [64, T] int32
        msg2 = np.concatenate([2 * msg, 2 * msg + 1], axis=0).astype(np.int32)
        # imt: [128 parts, 16*64]: chunk c at cols 64c, imt[p, 64c+m] =
        # images[sl][m, 128c+p]
        imc = images[sl].astype(np.float32)              # [64, 2048]
        imt = np.ascontiguousarray(
            imc.reshape(BS, 16, 128).transpose(2, 1, 0).reshape(128, 1024)
        ).astype(bfloat16)
        in_maps.append(
            {
                "m2p": M2p,
                "w2": W2,
                "msg2": msg2,
                "identb": identb,
                "imt": imt,
                "wimg": W_img_b,
                "whid": W_hid_b,
                "o2": o2,
                "bimg2": bimg2,
                "bhid2": bhid2,
            }
        )
    return in_maps


def kernel(images, embed_table, W_cell, b_cell, W_img, b_img, W_hid, b_hid,
           message):
    import sys
    if "/opt/trn_rl_repo" not in sys.path:
        sys.path.insert(0, "/opt/trn_rl_repo")
    from concourse.bass_utils import run_bass_kernel_spmd

    images = np.asarray(images, np.float32)
    embed_table = np.asarray(embed_table, np.float32)
    W_cell = np.asarray(W_cell, np.float32)
    b_cell = np.asarray(b_cell, np.float32)
    W_img = np.asarray(W_img, np.float32)
    b_img = np.asarray(b_img, np.float32)
    W_hid = np.asarray(W_hid, np.float32)
    b_hid = np.asarray(b_hid, np.float32)
    message = np.asarray(message, np.int32)

    n_steps = T
    if "nc" not in _CACHE or _CACHE.get("n_steps") != n_steps:
        _CACHE["nc"] = _build_nc(n_steps)
        _CACHE["n_steps"] = n_steps
    nc = _CACHE["nc"]

    in_maps = _host_prep(
        images, embed_table, W_cell, b_cell, W_img, b_img, W_hid, b_hid, message
    )
    res = run_bass_kernel_spmd(nc, in_maps, core_ids=list(range(NCORES)))
    results = res.results

    images_encoded = np.empty((B, OUT), np.float32)
    hidden_encoded = np.empty((B, OUT), np.float32)
    for core in range(NCORES):
        sl = slice(core * BS, (core + 1) * BS)
        oi = results[core]["oimg"]
        oh = results[core]["ohid"]
        images_encoded[sl, : OUT // 2] = oi[0:64]
        images_encoded[sl, OUT // 2 :] = oi[64:128]
        hidden_encoded[sl, : OUT // 2] = oh[0:64]
        hidden_encoded[sl, OUT // 2 :] = oh[64:128]
    return images_encoded, hidden_encoded
